# revision 17
# baseline (speedup 1.0000x reference)
"""Per-task adapter (MoE routing) on 8 TRN2 NeuronCores.

Strategy: expert-parallel. Host routes rows by task_id so core t gets all
rows with task t (capacity CAP=528; host fallback for overflow), each
core computes its expert's delta = silu(x @ Wd[t] + bd[t]) @ Wu[t], and
the host scatters deltas back, adding the f32 residual x and bu[t].

Device kernel is raw bacc (no TileContext) with hand-placed semaphores,
fp8-e4m3 I/O (weights pre-scaled by 16 on the host; the 1/16 is folded
into the silu activation scale, and the up output is descaled on host).

v2 layout: input is streamed on all THREE dynamic DMA rings concurrently
(sync/scalar HWDGE + gpsimd SWDGE; aggregate read ~300-400 GB/s vs ~260
on one ring), in need order with one cumulative semaphore per ring (ring
FIFO makes wait_ge(s, 16*k) == "first k DMAs done"):

  scalar: xA(ko 0-7), xA(ko 8-15), wu lo, wu hi
  sync:   wd, bd, xB(ko 0-3), xB(ko 4-7)
  gpsimd: xB(ko 8-11), xB(ko 12-15)

Phase A covers rows 0:256 plus the 16 tail rows (packed as down cols
256:272) so the packed tail-up matmuls and their outt cast/store retire
early instead of gating the end of the kernel; phase B is rows 256:512.

PSUM: bank0 = A accum (+ tail-up output after silu-A), bank1 = B accum,
banks 2-7 = three [128,1024] up slots; banks 0-1 are reclaimed as a 4th
slot for pair P5 once silu-B and the outt cast are done.  Casts
PSUM->SBUF fp8 alternate Vector (P0,P2,ot,P4,P6) / Scalar (P1,P3,P5,P7);
output DMAs stream per [128,1024] piece on gpsimd (cb0,cb1,outt) and
sync (cb2,cb3) behind each ring's input FIFO.

PE order: warmup dummies (HAM un-throttles ~3.4us in, right when the
first real matmul's input lands), down-A, up cb0, down-B ko0-7, up cb1,
down-B ko8-15, tail, up cb2, up cb3.
"""

import numpy as np
import ml_dtypes

N_TASKS = 8
SIZE = 2048
HID = 128
P = 128
KD = SIZE // P           # 16 contraction chunks for the down projection
CAP = 528                # per-core routed-row capacity (max seed-0 count is 527)
NA = 272                 # phase-A down cols: rows 0:256 + tail rows 512:528
NB = 256                 # phase-B down cols: rows 256:512
R = CAP - 512            # tail rows handled via partition-packed up matmuls
WSCALE = 16.0            # host pre-scale on Wd/Wu for fp8 dynamic range
ACT_FUNC = "Silu"
N_DUMMY = 12             # entry warmup matmuls (bridge to first real MM)

_NC = None


def _build_nc():
    import concourse.mybir as mybir
    from concourse import bacc

    dt = mybir.dt
    f8 = dt.float8e4
    act_fn = getattr(mybir.ActivationFunctionType, ACT_FUNC)
    import concourse.bass as cbass

    # The constructor tail emits a full all-engine EVSEM barrier (~3.5us on
    # silicon) guarding preamble state this kernel never reads. Every
    # cross-engine dependency below is explicitly semaphore-gated, so skip
    # the entry barrier; Block exit still emits its own.
    _orig_barrier = cbass.Bass.all_engine_barrier
    cbass.Bass.all_engine_barrier = lambda self, **kw: None
    try:
        nc = bacc.Bacc(
            "TRN2", debug=False, num_devices=N_TASKS, monotonic_sem_count=0
        )
    finally:
        cbass.Bass.all_engine_barrier = _orig_barrier

    xt = nc.dram_tensor("xt", [P, KD * CAP], f8, kind="ExternalInput")
    wdp = nc.dram_tensor("wdp", [P, KD * P], f8, kind="ExternalInput")
    wu = nc.dram_tensor("wu", [P, SIZE], f8, kind="ExternalInput")
    bdp = nc.dram_tensor("bdp", [P, 1], dt.float32, kind="ExternalInput")
    out = nc.dram_tensor("out", [512, SIZE], f8, kind="ExternalOutput")
    outt = nc.dram_tensor("outt", [P, 512], f8, kind="ExternalOutput")

    wd_sb = nc.alloc_sbuf_tensor("wd_sb", [P, KD, P], f8).ap()
    xa_sb = nc.alloc_sbuf_tensor("xa_sb", [P, KD, NA], f8).ap()
    xb_sb = nc.alloc_sbuf_tensor("xb_sb", [P, KD, NB], f8).ap()
    wu_sb = nc.alloc_sbuf_tensor("wu_sb", [P, SIZE], f8).ap()
    bd_sb = nc.alloc_sbuf_tensor("bd_sb", [P, 1], dt.float32).ap()
    h_sb = nc.alloc_sbuf_tensor("h_sb", [P, CAP], f8).ap()
    o_sb = nc.alloc_sbuf_tensor("o_sb", [P, 4, SIZE], f8).ap()
    ot_sb = nc.alloc_sbuf_tensor("ot_sb", [P, 512], f8).ap()
    dum_sb = nc.alloc_sbuf_tensor("dum_sb", [P, 512], f8).ap()
    dsc_sb = nc.alloc_sbuf_tensor("dsc_sb", [P, 1], dt.float32).ap()

    # All 8 PSUM banks as one tensor; 512-col bank-aligned slices.
    # bank0: phase-A down accum [*,0:NA], then tail-up output [*,0:512];
    # bank1: phase-B down accum [*,512:512+NB]; banks 2-7: up slots
    # S1/S2/S3; banks 0-1 re-used as S4 for pair P5.
    pall = nc.alloc_psum_tensor("pall", [P, 4096], dt.float32).ap()
    phA = pall[:, 0:NA]
    phB = pall[:, 512 : 512 + NB]
    # up pair p -> PSUM base col
    PAIR_BASE = {0: 1024, 1: 2048, 2: 3072, 3: 1024, 4: 2048, 5: 0, 6: 3072, 7: 1024}

    # one semaphore per input DMA: a wait of >=16 is only sound when the
    # sem is incremented by exactly that DMA (the 16 SDMA engines complete
    # their slices of consecutive ring DMAs in arbitrary interleave)
    sWd = nc.alloc_semaphore("sWd")
    sBd = nc.alloc_semaphore("sBd")
    sA1 = nc.alloc_semaphore("sA1")
    sA2 = nc.alloc_semaphore("sA2")
    sWua = nc.alloc_semaphore("sWua")
    sWub = nc.alloc_semaphore("sWub")
    sB1 = nc.alloc_semaphore("sB1")
    sB2 = nc.alloc_semaphore("sB2")
    sB3 = nc.alloc_semaphore("sB3")
    sB4 = nc.alloc_semaphore("sB4")
    sDNA = nc.alloc_semaphore("sDNA")
    sDNB = nc.alloc_semaphore("sDNB")
    sSil = nc.alloc_semaphore("sSil")
    sUP = nc.alloc_semaphore("sUP")
    sCV = nc.alloc_semaphore("sCV")
    sCS = nc.alloc_semaphore("sCS")
    sOUT = nc.alloc_semaphore("sOUT")
    sMS = nc.alloc_semaphore("sMS")
    sOUTG = nc.alloc_semaphore("sOUTG")  # gpsimd-ring output completions

    with nc.Block(no_gpsimd_drain=True) as block:

        @block.sync
        def _(sync):
            xbv = xt.ap()[:, KD * NA :].rearrange("p (ko c) -> p ko c", c=NB)
            sync.dma_start(
                wd_sb, wdp.ap().rearrange("p (ko m) -> p ko m", m=P)
            ).then_inc(sWd, 16)
            sync.dma_start(bd_sb, bdp.ap()).then_inc(sBd, 16)
            sync.dma_start(xb_sb[:, 0:4], xbv[:, 0:4]).then_inc(sB1, 16)
            sync.dma_start(xb_sb[:, 4:8], xbv[:, 4:8]).then_inc(sB2, 16)
            # output pieces: cb2, cb3 (queue FIFO already orders these
            # behind the input DMAs above, so no input-protect hold needed)
            for half, sem, cnt in (
                (0, sCV, 4),
                (1, sCS, 3),
            ):
                sync.wait_ge(sem, cnt)
                sync.dma_start(
                    out.ap()[2 * P : 3 * P, half * 1024 : (half + 1) * 1024],
                    o_sb[:, 2, half * 1024 : (half + 1) * 1024],
                ).then_inc(sOUT, 16)
            for half, sem, cnt in (
                (0, sCV, 5),
                (1, sCS, 4),
            ):
                sync.wait_ge(sem, cnt)
                sync.dma_start(
                    out.ap()[3 * P : 4 * P, half * 1024 : (half + 1) * 1024],
                    o_sb[:, 3, half * 1024 : (half + 1) * 1024],
                ).then_inc(sOUT, 16)
            sync.wait_ge(sOUT, 64)   # 4 sync-ring output DMAs
            sync.wait_ge(sOUTG, 80)  # 5 gpsimd-ring output DMAs

        @block.gpsimd
        def _(gpsimd):
            gpsimd.memset(dum_sb, 0).then_inc(sMS, 1)
            xbv = xt.ap()[:, KD * NA :].rearrange("p (ko c) -> p ko c", c=NB)
            gpsimd.dma_start(xb_sb[:, 8:12], xbv[:, 8:12]).then_inc(sB3, 16)
            gpsimd.dma_start(xb_sb[:, 12:16], xbv[:, 12:16]).then_inc(sB4, 16)
            # output pieces: cb0, cb1, outt
            for cb, half, sem, cnt in (
                (0, 0, sCV, 1),
                (0, 1, sCS, 1),
                (1, 0, sCV, 2),
                (1, 1, sCS, 2),
            ):
                gpsimd.wait_ge(sem, cnt)
                gpsimd.dma_start(
                    out.ap()[cb * P : (cb + 1) * P, half * 1024 : (half + 1) * 1024],
                    o_sb[:, cb, half * 1024 : (half + 1) * 1024],
                ).then_inc(sOUTG, 16)
            gpsimd.wait_ge(sCV, 3)
            gpsimd.dma_start(outt.ap(), ot_sb).then_inc(sOUTG, 16)

        @block.tensor
        def _(tensor):
            # warmup matmuls on uninitialized data bridge block entry to the
            # first x chunk so HAM un-throttles the PE to 2.4 GHz; every
            # later PSUM write uses start=True so garbage never leaks.
            def dummy_mm(n=512):
                tensor.matmul(
                    pall[:, 3072 : 3072 + n],
                    dum_sb[:, :P],
                    dum_sb[:, :n],
                    start=True,
                    stop=True,
                )

            def down(ph, x_sb, n, waits, sdone, jlo=0, jhi=8):
                DR = mybir.MatmulPerfMode.DoubleRow
                for j in range(jlo, jhi):
                    for sem, cnt in waits.get(j, ()):
                        tensor.wait_ge(sem, cnt)
                    ko = 2 * j
                    mm = tensor.matmul(
                        ph,
                        wd_sb[:, ko : ko + 2, :],
                        x_sb[:, ko : ko + 2, 0:n],
                        start=(j == 0),
                        stop=(j == 7),
                        perf_mode=DR,
                    )
                mm.then_inc(sdone, 1)

            # up matmul g: cb = g//4, ncx = g%4, pair p = g//2 -> PAIR_BASE.
            up_gates = {
                0: [(sSil, 1), (sWua, 16)],  # h cb0 + wu lo
                2: [(sWub, 16)],             # wu hi
                4: [(sSil, 2)],              # h cb1
                6: [(sCV, 1)],               # P3 -> S1 after V's cast of P0
                8: [(sSil, 4), (sCS, 1)],    # h cb2; P4 -> S2 after S's P1
                10: [(sCV, 3), (sSil, 5)],   # P5 -> banks 0-1 after outt cast
                12: [(sSil, 5), (sCV, 2)],   # h cb3; P6 -> S3 after V's P2
                14: [(sCS, 2)],              # P7 -> S1 after S's cast of P3
            }

            def up(g0, g1):
                for g in range(g0, g1):
                    cb, ncx = divmod(g, 4)
                    for sem, cnt in up_gates.get(g, ()):
                        tensor.wait_ge(sem, cnt)
                    base = PAIR_BASE[g // 2] + (g % 2) * 512
                    tensor.matmul(
                        pall[:, base : base + 512],
                        h_sb[:, cb * P : (cb + 1) * P],
                        wu_sb[:, ncx * 512 : (ncx + 1) * 512],
                        start=True,
                        stop=True,
                    ).then_inc(sUP, 1)

            tensor.wait_ge(sMS, 1)
            for _ in range(N_DUMMY):
                dummy_mm()
            down(phA, xa_sb, NA, {0: [(sA1, 16), (sWd, 16)], 4: [(sA2, 16)]}, sDNA)
            up(0, 4)                                   # cb0
            # down-B split at ko 8 so cb1's ups aren't stuck behind the
            # gpsimd ring's later chunks
            down(phB, xb_sb, NB, {0: [(sB1, 16)], 2: [(sB2, 16)]}, sDNB, 0, 4)
            up(4, 8)                                   # cb1
            down(phB, xb_sb, NB, {4: [(sB3, 16)], 6: [(sB4, 16)]}, sDNB, 4, 8)
            # tail rows 512:528 (= phase-A cols 256:272): 4 col-tiled MMs
            # pack [R x 2048] into bank0[:, :512]; chunk j at partitions 32j
            tensor.wait_ge(sSil, 3)  # bank0 free after silu-A (incl. tail)
            # zero-fill bank0 (dum_sb is zeroed) so the outt cast reads
            # defined values in the partition strips the tail MMs skip
            tensor.matmul(
                pall[:, 0:512], dum_sb[:, :P], dum_sb[:, :512], start=True, stop=True
            )
            for j in range(4):
                tensor.matmul(
                    pall[32 * j : 32 * j + R, 0:512],
                    h_sb[:, 512:CAP],
                    wu_sb[:, j * 512 : (j + 1) * 512],
                    start=True,
                    stop=True,
                    tile_position=(0, 32 * j),
                ).then_inc(sUP, 1)                      # sUP 9..12
            up(8, 12)                                  # cb2, sUP 13..16
            up(12, 16)                                 # cb3, sUP 17..20

        @block.scalar
        def _(scalar):
            xav = xt.ap()[:, : KD * NA].rearrange("p (ko c) -> p ko c", c=NA)
            scalar.dma_start(xa_sb[:, 0:8], xav[:, 0:8]).then_inc(sA1, 16)
            scalar.dma_start(xa_sb[:, 8:16], xav[:, 8:16]).then_inc(sA2, 16)
            scalar.dma_start(wu_sb[:, 0:1024], wu.ap()[:, 0:1024]).then_inc(sWua, 16)
            scalar.dma_start(wu_sb[:, 1024:2048], wu.ap()[:, 1024:2048]).then_inc(
                sWub, 16
            )
            # dummy silu first: loads silu_and_others (which contains copy)
            # during the DMA window -- one table set for the whole kernel
            scalar.wait_ge(sMS, 1)
            scalar.activation(dsc_sb, dum_sb[:, :1], act_fn)
            scalar.wait_ge(sBd, 16)
            scalar.wait_ge(sDNA, 1)
            scalar.activation(
                h_sb[:, 0:P], phA[:, 0:P], act_fn, bias=bd_sb, scale=1.0 / WSCALE
            ).then_inc(sSil, 1)
            scalar.activation(
                h_sb[:, P : 2 * P],
                phA[:, P : 2 * P],
                act_fn,
                bias=bd_sb,
                scale=1.0 / WSCALE,
            ).then_inc(sSil, 1)
            scalar.activation(
                h_sb[:, 512:CAP],
                phA[:, 2 * P : NA],
                act_fn,
                bias=bd_sb,
                scale=1.0 / WSCALE,
            ).then_inc(sSil, 1)
            # casts: pair p -> src PAIR_BASE[p], dst second half of cb p//2
            scalar.wait_ge(sUP, 4)
            scalar.copy(o_sb[:, 0, 1024:2048], pall[:, 2048:3072]).then_inc(sCS, 1)
            # silu-B before the P3 cast: cb2's ups are gated on it, while
            # the P7 consumer of the P3 cast comes later
            scalar.wait_ge(sDNB, 2)
            scalar.activation(
                h_sb[:, 2 * P : 3 * P],
                phB[:, 0:P],
                act_fn,
                bias=bd_sb,
                scale=1.0 / WSCALE,
            ).then_inc(sSil, 1)
            scalar.activation(
                h_sb[:, 3 * P : 4 * P],
                phB[:, P : 2 * P],
                act_fn,
                bias=bd_sb,
                scale=1.0 / WSCALE,
            ).then_inc(sSil, 1)
            scalar.wait_ge(sUP, 8)
            scalar.copy(o_sb[:, 1, 1024:2048], pall[:, 1024:2048]).then_inc(sCS, 1)
            scalar.wait_ge(sUP, 16)
            scalar.copy(o_sb[:, 2, 1024:2048], pall[:, 0:1024]).then_inc(sCS, 1)
            scalar.wait_ge(sUP, 20)
            scalar.copy(o_sb[:, 3, 1024:2048], pall[:, 1024:2048]).then_inc(sCS, 1)

        @block.vector
        def _(vector):
            # vector casts: first half of each cb + outt
            for wait, src, dst in (
                (2, 1024, (0, 0)),       # P0
                (6, 3072, (1, 0)),       # P2
                (12, None, None),        # outt (bank0 after tail MMs)
                (14, 2048, (2, 0)),      # P4
                (18, 3072, (3, 0)),      # P6
            ):
                vector.wait_ge(sUP, wait)
                if src is None:
                    vector.tensor_copy(ot_sb, pall[:, 0:512]).then_inc(sCV, 1)
                else:
                    cb, half = dst
                    vector.tensor_copy(
                        o_sb[:, cb, half * 1024 : half * 1024 + 1024],
                        pall[:, src : src + 1024],
                    ).then_inc(sCV, 1)

    nc.compile()
    return nc


def _get_nc():
    global _NC
    if _NC is None:
        _NC = _build_nc()
    return _NC


def _pack(xr):
    """[F, SIZE] f32 rows -> [P, KD*F] (p, ko-major, c) fp8-ready layout."""
    F = xr.shape[0]
    return xr.reshape(F, KD, P).transpose(2, 1, 0).reshape(P, KD * F)


def kernel(x, Wd, bd, Wu, bu, task_id):
    from concourse.bass_utils import run_bass_kernel_spmd

    x = np.asarray(x, dtype=np.float32)
    Wd = np.asarray(Wd, dtype=np.float32)
    bd = np.asarray(bd, dtype=np.float32)
    Wu = np.asarray(Wu, dtype=np.float32)
    bu = np.asarray(bu, dtype=np.float32)
    tid = np.asarray(task_id).astype(np.int64)

    f8 = ml_dtypes.float8_e4m3
    valid = tid >= 0
    t_clip = np.clip(tid, 0, N_TASKS - 1)

    in_maps = []
    rows_per_task = []
    overflow = []  # (task, rows) beyond CAP -> host fallback, keeps correctness
    for t in range(N_TASKS):
        rows = np.nonzero(valid & (t_clip == t))[0]
        if rows.size > CAP:
            overflow.append((t, rows[CAP:]))
            rows = rows[:CAP]
        rows_per_task.append(rows)

        xr = np.zeros((CAP, SIZE), dtype=np.float32)
        xr[: rows.size] = x[rows]
        xtp = np.empty((P, KD * CAP), dtype=np.float32)
        # phase A: rows 0:256 + tail rows 512:528 (as cols 256:272)
        xtp[:, : KD * NA] = _pack(np.concatenate([xr[:256], xr[512:CAP]], axis=0))
        xtp[:, KD * NA :] = _pack(xr[256:512])  # phase B
        wdpk = (
            (Wd[t] * WSCALE).reshape(KD, P, P).transpose(1, 0, 2).reshape(P, KD * P)
        )
        in_maps.append(
            {
                "xt": xtp.astype(f8),
                "wdp": np.ascontiguousarray(wdpk).astype(f8),
                "wu": (Wu[t] * WSCALE).astype(f8),
                "bdp": np.ascontiguousarray(bd[t].reshape(P, 1)),
            }
        )

    global _last_in_maps
    _last_in_maps = in_maps
    nc = _get_nc()
    res = run_bass_kernel_spmd(nc, in_maps, list(range(N_TASKS))).results

    out = x.copy()
    for t in range(N_TASKS):
        rows = rows_per_task[t]
        if rows.size == 0:
            continue
        o = np.asarray(res[t]["out"]).astype(np.float32)  # [512, SIZE]
        ot = np.asarray(res[t]["outt"]).astype(np.float32)  # [128, 512]
        tail = ot.reshape(4, 32, 512)[:, :R].transpose(1, 0, 2).reshape(R, SIZE)
        full = np.concatenate([o, tail], axis=0)
        delta = full[: rows.size] * (1.0 / WSCALE)
        out[rows] += delta + bu[t][None, :]
    for t, rows in overflow:
        hz = x[rows] @ Wd[t] + bd[t]
        h = hz / (1.0 + np.exp(-hz))
        out[rows] += h @ Wu[t] + bu[t]
    return out
